# revision 14
# baseline (speedup 1.0000x reference)
"""AttentionBlock (GroupNorm32 + QKV 8-head attention + proj + residual) on 8 TRN2 NeuronCores.

Sharding: pure data-parallel over batch B=8 - one batch element per core.

Schedule (per core), built to keep ScalarE (exp, the true bottleneck: 64
activations of 1024 elems each ~ 71us busy) saturated from ~20us to the end:
  - head: one packed consts DMA + one DMA per big tensor (DMA *issues* cost
    ~0.6us each on the sync queue). x is shipped once, in bf16 - the residual
    is folded into proj via identity-extended weights, so no f32 x is needed.
    PE warmup matmuls release the HAM clock gate; a dummy Exp preloads the
    ACT table. GroupNorm: per-tile sum(x) on DVE + sum(x^2) on the idle
    ScalarE (Square shares the exp table set), then one batched group-reduce
    + Newton-rsqrt on DVE. xn tiles interleave with k0's matmuls.
  - attention: blocks = (pair, n-half); per sm: QK (two row-tiled K=64
    matmuls) + one 1024-elem Exp; AV trails two sm globally so the evac
    chain can free its psum bank before the next block's AV needs it.
    Leftover qkv matmuls are pumped as filler units into PE slack; when they
    run dry, proj-m2 partial contractions and warm dummies keep the HAM
    clock gate open through the late pairs.
  - rowsums: the AV stationary operand is [v | ones-block]; psum partitions
    64:128 hold the softmax row-sum replicated at zero extra PE cost. Evac
    copies row-sums to a partition-0 SBUF tile (custom-DVE ops require
    base-partition-0 APs on HW), reciprocal_approx_fast, fused mul.
  - proj: contraction extended to 8 chunks with [pw | I] so psum accumulates
    pw@a + x directly; m0/m1 pre-contract the x-chunks + pairs 0-2 before the
    last evac; evacuation alternates ScalarE activation-Copy (bias=pb) and
    DVE tensor_scalar so the two engines drain the tail in parallel.
"""

import numpy as np
import ml_dtypes
from contextlib import ExitStack

import concourse.bass as bass
import concourse.tile as tile
from concourse import bacc, mybir
from concourse.bass_utils import run_bass_kernel_spmd

F32 = mybir.dt.float32
BF = mybir.dt.bfloat16
MULT = mybir.AluOpType.mult
ADD = mybir.AluOpType.add
AFT = mybir.ActivationFunctionType
AXX = mybir.AxisListType.X

C, T, H, CH = 512, 1024, 8, 64
NJ = C // 128          # 4 c-tiles
NTM = T // 128         # 8 t-tiles
NPJ = 8                # proj contraction chunks: [pw | I]
EXP_SCALE = float(CH) ** -0.5  # folded (q*s)*(k*s) scale, s = ch**-0.25
GN_N = 16 * T          # elements per group

BF_NP = ml_dtypes.bfloat16


def build_graph(enable_asserts: bool = False):
    nc = bacc.Bacc(
        "TRN2",
        target_bir_lowering=False,
        debug=False,
        enable_asserts=enable_asserts,
    )
    xbf_d = nc.dram_tensor("xbf", [C, T], BF, kind="ExternalInput").ap()
    wq_d = nc.dram_tensor("wq", [C, C], BF, kind="ExternalInput").ap()
    wk_d = nc.dram_tensor("wk", [C, C], BF, kind="ExternalInput").ap()
    wv_d = nc.dram_tensor("wv", [C, C], BF, kind="ExternalInput").ap()
    pw_d = nc.dram_tensor("pw", [2 * C, C], BF, kind="ExternalInput").ap()
    cp_d = nc.dram_tensor("cpack", [128, 28], F32, kind="ExternalInput").ap()
    gt8_d = nc.dram_tensor("gt8", [8, 128], F32, kind="ExternalInput").ap()
    out_d = nc.dram_tensor("out", [C, T], F32, kind="ExternalOutput").ap()

    with tile.TileContext(nc) as tc, ExitStack() as ctx:
        consts = ctx.enter_context(tc.tile_pool(name="consts", bufs=1))
        bigs = ctx.enter_context(tc.tile_pool(name="bigs", bufs=1))
        ewp = ctx.enter_context(tc.tile_pool(name="ewp", bufs=5))
        work = ctx.enter_context(tc.tile_pool(name="work", bufs=4))
        rinvp = ctx.enter_context(tc.tile_pool(name="rinvp", bufs=2))
        outp = ctx.enter_context(tc.tile_pool(name="outp", bufs=4))
        qk_ps = ctx.enter_context(tc.tile_pool(name="qk_ps", bufs=2, space="PSUM"))
        kv_ps = ctx.enter_context(tc.tile_pool(name="kv_ps", bufs=2, space="PSUM"))
        av_ps = ctx.enter_context(tc.tile_pool(name="av_ps", bufs=2, space="PSUM"))

        # ---- persistent sbuf tensors ----
        xbf = bigs.tile([128, NJ, T], BF)       # bf16 x (GN, qkv and residual)
        xn = bigs.tile([128, NJ, T], BF)        # groupnormed x
        q_sb = bigs.tile([128, NJ, T], BF)      # q rows (head-major)
        k_sb = bigs.tile([128, NJ, T], BF)      # k rows (head-major)
        vT2 = bigs.tile([128, NTM, H, 128], BF)  # v transposed | ones block
        a_sb = bigs.tile([128, NJ, T], BF)      # normalized attention output

        # ---- DMAs (issue order == priority; each issue ~0.6us on Sync) ----
        cpk = consts.tile([128, 28], F32)
        nc.sync.dma_start(cpk[:], cp_d[:])
        gt8_sb = consts.tile([8, 128], F32)
        nc.sync.dma_start(gt8_sb[:], gt8_d[:])
        gns_sb, gnb_sb = cpk[:, 0:4], cpk[:, 4:8]
        bq_sb, bk_sb, pb_sb = cpk[:, 8:12], cpk[:, 12:16], cpk[:, 16:20]
        g8_sb = cpk[:, 20:28]

        def stacked(dram, ntiles, width):
            return bass.AP(tensor=dram.tensor, offset=0,
                           ap=[[width, 128], [128 * width, ntiles], [1, width]])

        for j in range(NJ):
            nc.sync.dma_start(xbf[:, j, :], xbf_d[j * 128:(j + 1) * 128, :])
        wk_sb = consts.tile([128, NJ, C], BF)
        wq_sb = consts.tile([128, NJ, C], BF)
        wv_sb = consts.tile([128, NJ, C], BF)
        pw_sb = consts.tile([128, NPJ, C], BF)
        nc.sync.dma_start(wk_sb[:], stacked(wk_d, NJ, C))
        nc.sync.dma_start(wq_sb[:], stacked(wq_d, NJ, C))
        nc.sync.dma_start(wv_sb[:], stacked(wv_d, NJ, C))
        nc.sync.dma_start(pw_sb[:], stacked(pw_d, NPJ, C))

        # ---- memsets + ACT exp-table preload (runs during DMA wait) ----
        zero_sb = consts.tile([128, 1], F32)
        nc.vector.memset(zero_sb[:], 0.0)
        warm_sb = consts.tile([128, 512], BF)
        nc.vector.memset(warm_sb[:], 0.125)
        nc.vector.memset(vT2[:, :, :, CH:128], 1.0)   # ones block for row-sums
        dume = consts.tile([128, 1], F32)
        nc.scalar.activation(dume[:], zero_sb[:], AFT.Exp, bias=zero_sb[:], scale=1.0)

        # ---- PE warmup: release the HAM clock gate before real matmuls ----
        def warm_mm(n=1):
            for _ in range(n):
                wps = kv_ps.tile([128, 512], F32, tag="kv", name="wps")
                nc.tensor.matmul(wps[:], warm_sb[:, 0:128], warm_sb[:],
                                 start=True, stop=True)

        def warm_dep(rhs):
            wps = kv_ps.tile([128, 512], F32, tag="kv", name="wpsd")
            nc.tensor.matmul(wps[:, 0:rhs.shape[-1]], warm_sb[:, 0:128], rhs,
                             start=True, stop=True)
        warm_mm(20)

        # ---- GroupNorm stats: sum(x) on DVE, sum(x^2) on ScalarE ----
        stats_sb = consts.tile([128, 8], F32)   # sum(x) j=0..3 | sum(x^2) j=0..3
        sqs = consts.tile([128, T], BF)         # Square scratch output
        for j in range(NJ):
            nc.vector.tensor_reduce(stats_sb[:, j:j + 1], xbf[:, j, :], AXX, ADD)
            nc.scalar.activation(sqs[:], xbf[:, j, :], AFT.Square,
                                 accum_out=stats_sb[:, 4 + j:5 + j])
            warm_dep(sqs[0:128, 0:512])

        # ---- batched group-reduce + Newton rsqrt + affine ----
        ps_st = kv_ps.tile([128, 512], F32, tag="kv")
        nc.tensor.matmul(ps_st[0:8, 0:8], g8_sb, stats_sb[:], start=True, stop=True)
        stg = work.tile([8, 16], F32, tag="stg")     # mean(0:4)|var(4:8)|t1|t2
        bcin = work.tile([8, 8], F32, tag="bcin")    # mean | rinv
        # g8 host values are pre-scaled by 1/GN_N, so ps_st already holds means
        nc.vector.tensor_copy(stg[:, 0:8], ps_st[0:8, 0:8])
        nc.vector.tensor_mul(stg[:, 8:12], stg[:, 0:4], stg[:, 0:4])
        nc.vector.scalar_tensor_tensor(stg[:, 4:8], stg[:, 8:12], -1.0, stg[:, 4:8],
                                       op0=MULT, op1=ADD)   # var
        nc.vector.tensor_scalar(bcin[:, 4:8], stg[:, 4:8], -0.5, 1.5, op0=MULT, op1=ADD)
        warm_mm(1)
        for _ in range(1):  # Newton: y = y*(1.5 - 0.5*var*y^2)
            nc.vector.tensor_mul(stg[:, 8:12], stg[:, 4:8], bcin[:, 4:8])
            nc.vector.tensor_mul(stg[:, 12:16], stg[:, 8:12], bcin[:, 4:8])
            nc.vector.tensor_scalar(stg[:, 12:16], stg[:, 12:16], -0.5, 1.5, op0=MULT, op1=ADD)
            nc.vector.tensor_mul(bcin[:, 4:8], bcin[:, 4:8], stg[:, 12:16])
        nc.vector.tensor_copy(bcin[:, 0:4], stg[:, 0:4])
        ps_pp = kv_ps.tile([128, 512], F32, tag="kv")
        nc.tensor.matmul(ps_pp[0:128, 0:8], gt8_sb[:], bcin[:], start=True, stop=True)
        ab = consts.tile([128, 2, NJ], F32)   # scale | shift per c-tile
        nc.vector.tensor_mul(ab[:, 0, :], ps_pp[0:128, 4:8], gns_sb)
        t1b = work.tile([128, 4], F32, tag="t1b")
        nc.vector.tensor_mul(t1b[:], ps_pp[0:128, 0:4], ab[:, 0, :])
        nc.vector.tensor_sub(ab[:, 1, :], gnb_sb, t1b[:])

        # xn_j interleaved with k0's j-matmuls
        psk0 = kv_ps.tile([128, 512], F32, tag="kv", name="psk0")
        psk1 = kv_ps.tile([128, 512], F32, tag="kv", name="psk1")
        for j in range(NJ):
            nc.vector.tensor_scalar(xn[:, j, :], xbf[:, j, :],
                                    ab[:, 0, j:j + 1], ab[:, 1, j:j + 1],
                                    op0=MULT, op1=ADD)
            for n in range(2):
                nc.tensor.matmul([psk0, psk1][n][:],
                                 wk_sb[:, j, 0:128],
                                 xn[:, j, 512 * n:512 * (n + 1)],
                                 start=(j == 0), stop=(j == NJ - 1))
        for n in range(2):
            nc.vector.tensor_scalar(k_sb[:, 0, 512 * n:512 * (n + 1)], [psk0, psk1][n][:],
                                    bk_sb[:, 0:1], None, op0=ADD)

        # ---- filler units: a few matmuls + 1 evac each, pumped into PE slack ----
        def kq_unit(w_sb, b_sb, dst, m, n):
            def emit():
                ps = kv_ps.tile([128, 512], F32, tag="kv", name="ps_kq")
                for j in range(NJ):
                    nc.tensor.matmul(ps[:],
                                     w_sb[:, j, 128 * m:128 * (m + 1)],
                                     xn[:, j, 512 * n:512 * (n + 1)],
                                     start=(j == 0), stop=(j == NJ - 1))
                nc.vector.tensor_scalar(dst[:, m, 512 * n:512 * (n + 1)], ps[:],
                                        b_sb[:, m:m + 1], None, op0=ADD)
            return emit

        def v_unit(tm, h0, h1):
            def emit():
                w = (h1 - h0) * CH
                ps = kv_ps.tile([128, 512], F32, tag="kv", name="ps_v")
                for j in range(NJ):
                    nc.tensor.matmul(ps[:, 0:w],
                                     xn[:, j, 128 * tm:128 * (tm + 1)],
                                     wv_sb[:, j, CH * h0:CH * h1],
                                     start=(j == 0), stop=(j == NJ - 1))
                nc.vector.tensor_copy(vT2[:, tm, h0:h1, 0:CH],
                                      ps[:, 0:w].rearrange("p (h c) -> p h c", c=CH))
            return emit

        # v split per head-pair: heads 2p:2p+2 are first needed by pair p.
        fillers = []
        for tm in range(2, NTM):
            fillers.append(v_unit(tm, 0, 2))
        for n in range(2):
            fillers.append(kq_unit(wk_sb, bk_sb, k_sb, 1, n))
        for n in range(2):
            fillers.append(kq_unit(wq_sb, bq_sb, q_sb, 1, n))
        for tm in range(NTM):
            fillers.append(v_unit(tm, 2, 4))
        for n in range(2):
            fillers.append(kq_unit(wk_sb, bk_sb, k_sb, 2, n))
        for n in range(2):
            fillers.append(kq_unit(wq_sb, bq_sb, q_sb, 2, n))
        for tm in range(NTM):
            fillers.append(v_unit(tm, 4, 6))
        for tm in range(NTM):
            fillers.append(v_unit(tm, 6, H))
        for n in range(2):
            fillers.append(kq_unit(wk_sb, bk_sb, k_sb, 3, n))
        for n in range(2):
            fillers.append(kq_unit(wq_sb, bq_sb, q_sb, 3, n))

        # proj m=2 accumulates as a late filler: x-chunks any time, a-chunks
        # once the producing pair's evac has been emitted.
        psm2 = [None, None]

        def m2_unit(chunks, first):
            def emit():
                if first:
                    psm2[0] = kv_ps.tile([128, 512], F32, tag="kv", name="psm2a")
                    psm2[1] = kv_ps.tile([128, 512], F32, tag="kv", name="psm2b")
                for nh in range(2):
                    for jj in chunks:
                        rhs = (xbf if jj >= NJ else a_sb)
                        nc.tensor.matmul(psm2[nh][:],
                                         pw_sb[:, jj, 256:384],
                                         rhs[:, jj % NJ, 512 * nh:512 * (nh + 1)],
                                         start=(first and jj == chunks[0]), stop=False)
            return emit

        late = [(44, m2_unit([4, 5], True)), (47, m2_unit([6, 7], False)),
                (50, m2_unit([0], False)), (54, m2_unit([1], False)),
                (58, m2_unit([2], False))]

        def pump(slot):
            if fillers:
                fillers.pop(0)()
            elif late and slot >= late[0][0]:
                late.pop(0)[1]()
            elif slot % 2 and slot < 44:
                warm_mm(1)

        # ---- q0 / first v tiles (k0 was interleaved with xn above) ----
        for n in range(2):
            kq_unit(wq_sb, bq_sb, q_sb, 0, n)()
        v_unit(0, 0, 2)()
        v_unit(1, 0, 2)()

        # ---- attention: software-pipelined (AV trails two sm globally) ----
        def emit_qk(p, n, sm):
            psw = qk_ps.tile([128, T], F32, tag="qk", name="psw")
            nc.tensor.matmul(psw[:, 0:512],
                             k_sb[0:64, p, 128 * sm:128 * (sm + 1)],
                             q_sb[0:64, p, 512 * n:512 * (n + 1)],
                             start=True, stop=True, tile_position=(0, 0))
            nc.tensor.matmul(psw[:, 512:1024],
                             k_sb[64:128, p, 128 * sm:128 * (sm + 1)],
                             q_sb[64:128, p, 512 * n:512 * (n + 1)],
                             start=True, stop=True, tile_position=(64, 0))
            ew = ewp.tile([128, 2, 512], BF, tag="ew", name="ew")
            nc.scalar.activation(ew[:], psw[:].rearrange("p (u t) -> p u t", u=2),
                                 AFT.Exp, bias=zero_sb[:], scale=EXP_SCALE)
            return ew

        def emit_evac(p, n, psa):
            # custom-DVE ops need partition-0 based APs on HW: stage row-sums
            for u in range(2):
                rs = rinvp.tile([64, 512], F32, tag="rs", name="rs")
                nc.vector.tensor_copy(rs[:], psa[u][64:128, :])
                rinv = rinvp.tile([64, 512], F32, tag="rinv", name="rinv")
                nc.vector.reciprocal_approx_fast(rinv[:], rs[:])
                nc.vector.tensor_mul(a_sb[64 * u:64 * (u + 1), p, 512 * n:512 * (n + 1)],
                                     psa[u][0:CH, :], rinv[:])

        blocks = [(p, n) for p in range(NJ) for n in range(2)]
        bpsa = {}
        pend = []

        def drain(auto_evac=True):
            bi, p, n, sm, ew = pend.pop(0)
            for u in range(2):
                nc.tensor.matmul(bpsa[bi][u][:],
                                 vT2[:, sm, 2 * p + u, :],
                                 ew[:, u, :],
                                 start=(sm == 0), stop=(sm == NTM - 1))
            if sm == NTM - 1 and auto_evac:
                emit_evac(p, n, bpsa[bi])
                return True
            return False

        for bi, (p, n) in enumerate(blocks):
            psa0 = av_ps.tile([128, 512], F32, tag="av", name="psa0")
            psa1 = av_ps.tile([128, 512], F32, tag="av", name="psa1")
            bpsa[bi] = [psa0, psa1]
            for sm in range(NTM):
                slot = 8 * bi + sm
                ew = emit_qk(p, n, sm)
                pend.append((bi, p, n, sm, ew))
                evd = False
                if len(pend) > 2:
                    evd = drain()
                if not evd:
                    pump(slot)

        # ---- tail ----
        drain()                      # AV(last, 6)
        lbi, lp, ln_, lsm, _lew = pend[0]
        drain(auto_evac=False)       # AV(last, 7)
        # m0 / m1: all chunks except j=3 (the last pair's a-tiles)
        psp0 = qk_ps.tile([128, T], F32, tag="qk", name="psp0")
        psp1 = qk_ps.tile([128, T], F32, tag="qk", name="psp1")
        pre = [4, 5, 6, 7, 0, 1, 2]
        for m in range(2):
            for nh in range(2):
                for jj in pre:
                    rhs = (xbf if jj >= NJ else a_sb)
                    nc.tensor.matmul([psp0, psp1][m][:, 512 * nh:512 * (nh + 1)],
                                     pw_sb[:, jj, 128 * m:128 * (m + 1)],
                                     rhs[:, jj % NJ, 512 * nh:512 * (nh + 1)],
                                     start=(jj == pre[0]), stop=False)
        emit_evac(lp, ln_, bpsa[lbi])
        for m in range(2):
            for nh in range(2):
                nc.tensor.matmul([psp0, psp1][m][:, 512 * nh:512 * (nh + 1)],
                                 pw_sb[:, 3, 128 * m:128 * (m + 1)],
                                 a_sb[:, 3, 512 * nh:512 * (nh + 1)],
                                 start=False, stop=True)
        for nh in range(2):
            nc.tensor.matmul(psm2[nh][:],
                             pw_sb[:, 3, 256:384],
                             a_sb[:, 3, 512 * nh:512 * (nh + 1)],
                             start=False, stop=True)
        # evacuations: ScalarE (activation Copy + per-partition bias) and DVE
        # (tensor_scalar) drain in parallel; m3 runs on the freed kv banks.
        osb0 = outp.tile([128, T], F32, tag="osb", name="osb0")
        nc.scalar.activation(osb0[:], psp0[:], AFT.Identity, bias=pb_sb[:, 0:1], scale=1.0)
        nc.sync.dma_start(out_d[0:128, :], osb0[:])
        osb1 = outp.tile([128, T], F32, tag="osb", name="osb1")
        nc.vector.tensor_scalar(osb1[:], psp1[:], pb_sb[:, 1:2], None, op0=ADD)
        nc.sync.dma_start(out_d[128:256, :], osb1[:])
        psm3 = [av_ps.tile([128, 512], F32, tag="av", name="psm3a"),
                av_ps.tile([128, 512], F32, tag="av", name="psm3b")]
        for nh in range(2):
            for jj in range(NPJ):
                rhs = (xbf if jj >= NJ else a_sb)
                nc.tensor.matmul(psm3[nh][:],
                                 pw_sb[:, jj, 384:512],
                                 rhs[:, jj % NJ, 512 * nh:512 * (nh + 1)],
                                 start=(jj == 0), stop=(jj == NPJ - 1))
        for nh in range(2):
            osb2 = outp.tile([128, 512], F32, tag="osbh", name="osb2")
            nc.scalar.activation(osb2[:], psm2[nh][:], AFT.Identity, bias=pb_sb[:, 2:3], scale=1.0)
            nc.sync.dma_start(out_d[256:384, 512 * nh:512 * (nh + 1)], osb2[:])
        for nh in range(2):
            osb3 = outp.tile([128, 512], F32, tag="osbh", name="osb3")
            nc.vector.tensor_scalar(osb3[:], psm3[nh][:], pb_sb[:, 3:4], None, op0=ADD)
            nc.sync.dma_start(out_d[384:512, 512 * nh:512 * (nh + 1)], osb3[:])

    nc.compile()
    return nc


_NC_CACHE = {}


def get_nc():
    if "nc" not in _NC_CACHE:
        _NC_CACHE["nc"] = build_graph()
    return _NC_CACHE["nc"]


def make_in_maps(x, norm_scale, norm_bias, qkv_w, qkv_b, proj_w, proj_b):
    x = np.asarray(x, dtype=np.float32)
    B = x.shape[0]
    qr = np.asarray(qkv_w, np.float32).reshape(H, 3, CH, C)
    wq = np.ascontiguousarray(qr[:, 0].reshape(C, C).T).astype(BF_NP)
    wk = np.ascontiguousarray(qr[:, 1].reshape(C, C).T).astype(BF_NP)
    wv = np.ascontiguousarray(qr[:, 2].reshape(C, C).T).astype(BF_NP)
    br = np.asarray(qkv_b, np.float32).reshape(H, 3, CH)
    bq = np.ascontiguousarray(br[:, 0].reshape(C))
    bk = np.ascontiguousarray(br[:, 1].reshape(C))
    bv = np.ascontiguousarray(br[:, 2].reshape(C))
    pw_f = np.asarray(proj_w, np.float32)
    # residual folded in: [pw | I] so proj psum accumulates pw@a + x
    pw2 = np.ascontiguousarray(
        np.concatenate([pw_f.T, np.eye(C, dtype=np.float32)], axis=0)).astype(BF_NP)
    # v bias folded through proj: h = pw @ (a + bv) + pb = pw @ a + (pw@bv + pb)
    pb2 = np.asarray(proj_b, np.float32) + pw_f @ bv
    g8 = np.zeros((128, 8), np.float32)
    g8[np.arange(128), np.arange(128) // 16] = 1.0
    gt8 = np.ascontiguousarray(g8.T)
    g8s = g8 * np.float32(1.0 / GN_N)   # fold the 1/N of the group mean into g8
    cpack = np.zeros((128, 28), np.float32)
    cpack[:, 0:4] = np.asarray(norm_scale, np.float32).reshape(NJ, 128).T
    cpack[:, 4:8] = np.asarray(norm_bias, np.float32).reshape(NJ, 128).T
    cpack[:, 8:12] = bq.reshape(NJ, 128).T
    cpack[:, 12:16] = bk.reshape(NJ, 128).T
    cpack[:, 16:20] = pb2.reshape(NJ, 128).T
    cpack[:, 20:28] = g8s
    shared = dict(wq=wq, wk=wk, wv=wv, pw=pw2,
                  cpack=np.ascontiguousarray(cpack),
                  gt8=gt8)
    in_maps = []
    for i in range(B):
        m = dict(shared)
        m["xbf"] = np.ascontiguousarray(x[i].reshape(C, T).astype(BF_NP))
        in_maps.append(m)
    return in_maps


def kernel(x, norm_scale, norm_bias, qkv_w, qkv_b, proj_w, proj_b):
    x = np.asarray(x, dtype=np.float32)
    B, Cc, Hh, Ww = x.shape
    nc = get_nc()
    in_maps = make_in_maps(x, norm_scale, norm_bias, qkv_w, qkv_b, proj_w, proj_b)
    res = run_bass_kernel_spmd(nc, in_maps, core_ids=list(range(B)))
    out = np.stack([res.results[i]["out"] for i in range(B)])
    return out.reshape(B, Cc, Hh, Ww).astype(np.float32)


# revision 15
# speedup vs baseline: 1.0056x; 1.0056x over previous
"""AttentionBlock (GroupNorm32 + QKV 8-head attention + proj + residual) on 8 TRN2 NeuronCores.

Sharding: pure data-parallel over batch B=8 - one batch element per core.

Schedule (per core), built to keep ScalarE (exp, the true bottleneck: 64
activations of 1024 elems each ~ 71us busy) saturated from ~20us to the end:
  - head: one packed consts DMA + one DMA per big tensor (DMA *issues* cost
    ~0.6us each on the sync queue). x is shipped once, in bf16 - the residual
    is folded into proj via identity-extended weights, so no f32 x is needed.
    PE warmup matmuls release the HAM clock gate; a dummy Exp preloads the
    ACT table. GroupNorm: per-tile sum(x) on DVE + sum(x^2) on the idle
    ScalarE (Square shares the exp table set), then one batched group-reduce
    + Newton-rsqrt on DVE. xn tiles interleave with k0's matmuls.
  - attention: blocks = (pair, n-half); per sm: QK (two row-tiled K=64
    matmuls) + one 1024-elem Exp; AV trails two sm globally so the evac
    chain can free its psum bank before the next block's AV needs it.
    Leftover qkv matmuls are pumped as filler units into PE slack; when they
    run dry, proj-m2 partial contractions and warm dummies keep the HAM
    clock gate open through the late pairs.
  - rowsums: the AV stationary operand is [v | ones-block]; psum partitions
    64:128 hold the softmax row-sum replicated at zero extra PE cost. Evac
    copies row-sums to a partition-0 SBUF tile (custom-DVE ops require
    base-partition-0 APs on HW), reciprocal_approx_fast, fused mul.
  - proj: contraction extended to 8 chunks with [pw | I] so psum accumulates
    pw@a + x directly; m0/m1 pre-contract the x-chunks + pairs 0-2 before the
    last evac; evacuation alternates ScalarE activation-Copy (bias=pb) and
    DVE tensor_scalar so the two engines drain the tail in parallel.
"""

import numpy as np
import ml_dtypes
from contextlib import ExitStack

import concourse.bass as bass
import concourse.tile as tile
from concourse import bacc, mybir
from concourse.bass_utils import run_bass_kernel_spmd

F32 = mybir.dt.float32
BF = mybir.dt.bfloat16
MULT = mybir.AluOpType.mult
ADD = mybir.AluOpType.add
AFT = mybir.ActivationFunctionType
AXX = mybir.AxisListType.X

C, T, H, CH = 512, 1024, 8, 64
NJ = C // 128          # 4 c-tiles
NTM = T // 128         # 8 t-tiles
NPJ = 8                # proj contraction chunks: [pw | I]
EXP_SCALE = float(CH) ** -0.5  # folded (q*s)*(k*s) scale, s = ch**-0.25
GN_N = 16 * T          # elements per group

BF_NP = ml_dtypes.bfloat16


def build_graph(enable_asserts: bool = False):
    nc = bacc.Bacc(
        "TRN2",
        target_bir_lowering=False,
        debug=False,
        enable_asserts=enable_asserts,
    )
    xbf_d = nc.dram_tensor("xbf", [C, T], BF, kind="ExternalInput").ap()
    wq_d = nc.dram_tensor("wq", [C, C], BF, kind="ExternalInput").ap()
    wk_d = nc.dram_tensor("wk", [C, C], BF, kind="ExternalInput").ap()
    wv_d = nc.dram_tensor("wv", [C, C], BF, kind="ExternalInput").ap()
    pw_d = nc.dram_tensor("pw", [2 * C, C], BF, kind="ExternalInput").ap()
    cp_d = nc.dram_tensor("cpack", [128, 28], F32, kind="ExternalInput").ap()
    gt8_d = nc.dram_tensor("gt8", [8, 128], F32, kind="ExternalInput").ap()
    out_d = nc.dram_tensor("out", [C, T], F32, kind="ExternalOutput").ap()

    with tile.TileContext(nc) as tc, ExitStack() as ctx:
        consts = ctx.enter_context(tc.tile_pool(name="consts", bufs=1))
        bigs = ctx.enter_context(tc.tile_pool(name="bigs", bufs=1))
        ewp = ctx.enter_context(tc.tile_pool(name="ewp", bufs=5))
        work = ctx.enter_context(tc.tile_pool(name="work", bufs=4))
        rinvp = ctx.enter_context(tc.tile_pool(name="rinvp", bufs=2))
        outp = ctx.enter_context(tc.tile_pool(name="outp", bufs=4))
        qk_ps = ctx.enter_context(tc.tile_pool(name="qk_ps", bufs=2, space="PSUM"))
        kv_ps = ctx.enter_context(tc.tile_pool(name="kv_ps", bufs=2, space="PSUM"))
        av_ps = ctx.enter_context(tc.tile_pool(name="av_ps", bufs=2, space="PSUM"))

        # ---- persistent sbuf tensors ----
        xbf = bigs.tile([128, NJ, T], BF)       # bf16 x (GN, qkv and residual)
        xn = bigs.tile([128, NJ, T], BF)        # groupnormed x
        q_sb = bigs.tile([128, NJ, T], BF)      # q rows (head-major)
        k_sb = bigs.tile([128, NJ, T], BF)      # k rows (head-major)
        vT2 = bigs.tile([128, NTM, H, 128], BF)  # v transposed | ones block
        a_sb = bigs.tile([128, NJ, T], BF)      # normalized attention output

        # ---- DMAs (issue order == priority; each issue ~0.6us on Sync) ----
        for j in range(NJ):
            nc.sync.dma_start(xbf[:, j, :], xbf_d[j * 128:(j + 1) * 128, :])
        cpk = consts.tile([128, 28], F32)
        nc.sync.dma_start(cpk[:], cp_d[:])
        gt8_sb = consts.tile([8, 128], F32)
        nc.sync.dma_start(gt8_sb[:], gt8_d[:])
        gns_sb, gnb_sb = cpk[:, 0:4], cpk[:, 4:8]
        bq_sb, bk_sb, pb_sb = cpk[:, 8:12], cpk[:, 12:16], cpk[:, 16:20]
        g8_sb = cpk[:, 20:28]

        def stacked(dram, ntiles, width):
            return bass.AP(tensor=dram.tensor, offset=0,
                           ap=[[width, 128], [128 * width, ntiles], [1, width]])
        wk_sb = consts.tile([128, NJ, C], BF)
        wq_sb = consts.tile([128, NJ, C], BF)
        wv_sb = consts.tile([128, NJ, C], BF)
        pw_sb = consts.tile([128, NPJ, C], BF)
        nc.sync.dma_start(wk_sb[:], stacked(wk_d, NJ, C))
        nc.sync.dma_start(wq_sb[:], stacked(wq_d, NJ, C))
        nc.sync.dma_start(wv_sb[:], stacked(wv_d, NJ, C))
        nc.sync.dma_start(pw_sb[:], stacked(pw_d, NPJ, C))

        # ---- memsets + ACT exp-table preload (runs during DMA wait) ----
        zero_sb = consts.tile([128, 1], F32)
        nc.vector.memset(zero_sb[:], 0.0)
        warm_sb = consts.tile([128, 512], BF)
        nc.vector.memset(warm_sb[:], 0.125)
        nc.vector.memset(vT2[:, :, :, CH:128], 1.0)   # ones block for row-sums
        dume = consts.tile([128, 1], F32)
        nc.scalar.activation(dume[:], zero_sb[:], AFT.Exp, bias=zero_sb[:], scale=1.0)

        # ---- PE warmup: release the HAM clock gate before real matmuls ----
        def warm_mm(n=1):
            for _ in range(n):
                wps = kv_ps.tile([128, 512], F32, tag="kv", name="wps")
                nc.tensor.matmul(wps[:], warm_sb[:, 0:128], warm_sb[:],
                                 start=True, stop=True)

        def warm_dep(rhs):
            wps = kv_ps.tile([128, 512], F32, tag="kv", name="wpsd")
            nc.tensor.matmul(wps[:, 0:rhs.shape[-1]], warm_sb[:, 0:128], rhs,
                             start=True, stop=True)
        warm_mm(20)

        # ---- GroupNorm stats: sum(x) on DVE, sum(x^2) on ScalarE ----
        stats_sb = consts.tile([128, 8], F32)   # sum(x) j=0..3 | sum(x^2) j=0..3
        sqs = consts.tile([128, T], BF)         # Square scratch output
        for j in range(NJ):
            nc.vector.tensor_reduce(stats_sb[:, j:j + 1], xbf[:, j, :], AXX, ADD)
            nc.scalar.activation(sqs[:], xbf[:, j, :], AFT.Square,
                                 accum_out=stats_sb[:, 4 + j:5 + j])
            warm_dep(sqs[0:128, 0:512])

        # ---- batched group-reduce + Newton rsqrt + affine ----
        ps_st = kv_ps.tile([128, 512], F32, tag="kv")
        nc.tensor.matmul(ps_st[0:8, 0:8], g8_sb, stats_sb[:], start=True, stop=True)
        stg = work.tile([8, 16], F32, tag="stg")     # mean(0:4)|var(4:8)|t1|t2
        bcin = work.tile([8, 8], F32, tag="bcin")    # mean | rinv
        # g8 host values are pre-scaled by 1/GN_N, so ps_st already holds means
        nc.vector.tensor_copy(stg[:, 0:8], ps_st[0:8, 0:8])
        nc.vector.tensor_mul(stg[:, 8:12], stg[:, 0:4], stg[:, 0:4])
        nc.vector.scalar_tensor_tensor(stg[:, 4:8], stg[:, 8:12], -1.0, stg[:, 4:8],
                                       op0=MULT, op1=ADD)   # var
        nc.vector.tensor_scalar(bcin[:, 4:8], stg[:, 4:8], -0.5, 1.5, op0=MULT, op1=ADD)
        warm_mm(1)
        for _ in range(1):  # Newton: y = y*(1.5 - 0.5*var*y^2)
            nc.vector.tensor_mul(stg[:, 8:12], stg[:, 4:8], bcin[:, 4:8])
            nc.vector.tensor_mul(stg[:, 12:16], stg[:, 8:12], bcin[:, 4:8])
            nc.vector.tensor_scalar(stg[:, 12:16], stg[:, 12:16], -0.5, 1.5, op0=MULT, op1=ADD)
            nc.vector.tensor_mul(bcin[:, 4:8], bcin[:, 4:8], stg[:, 12:16])
        nc.vector.tensor_copy(bcin[:, 0:4], stg[:, 0:4])
        # dep-injected warm burst: re-release the HAM clock gate right before
        # k0 (sparse single matmuls during GN are not enough to hold it)
        nc.vector.tensor_copy(warm_sb[0:1, 0:1], bcin[0:1, 0:1])
        warm_mm(10)
        ps_pp = kv_ps.tile([128, 512], F32, tag="kv")
        nc.tensor.matmul(ps_pp[0:128, 0:8], gt8_sb[:], bcin[:], start=True, stop=True)
        ab = consts.tile([128, 2, NJ], F32)   # scale | shift per c-tile
        nc.vector.tensor_mul(ab[:, 0, :], ps_pp[0:128, 4:8], gns_sb)
        t1b = work.tile([128, 4], F32, tag="t1b")
        nc.vector.tensor_mul(t1b[:], ps_pp[0:128, 0:4], ab[:, 0, :])
        nc.vector.tensor_sub(ab[:, 1, :], gnb_sb, t1b[:])

        # xn_j interleaved with k0's j-matmuls
        psk0 = kv_ps.tile([128, 512], F32, tag="kv", name="psk0")
        psk1 = kv_ps.tile([128, 512], F32, tag="kv", name="psk1")
        for j in range(NJ):
            nc.vector.tensor_scalar(xn[:, j, :], xbf[:, j, :],
                                    ab[:, 0, j:j + 1], ab[:, 1, j:j + 1],
                                    op0=MULT, op1=ADD)
            for n in range(2):
                nc.tensor.matmul([psk0, psk1][n][:],
                                 wk_sb[:, j, 0:128],
                                 xn[:, j, 512 * n:512 * (n + 1)],
                                 start=(j == 0), stop=(j == NJ - 1))
        for n in range(2):
            nc.vector.tensor_scalar(k_sb[:, 0, 512 * n:512 * (n + 1)], [psk0, psk1][n][:],
                                    bk_sb[:, 0:1], None, op0=ADD)

        # ---- filler units: a few matmuls + 1 evac each, pumped into PE slack ----
        def kq_unit(w_sb, b_sb, dst, m, n):
            def emit():
                ps = kv_ps.tile([128, 512], F32, tag="kv", name="ps_kq")
                for j in range(NJ):
                    nc.tensor.matmul(ps[:],
                                     w_sb[:, j, 128 * m:128 * (m + 1)],
                                     xn[:, j, 512 * n:512 * (n + 1)],
                                     start=(j == 0), stop=(j == NJ - 1))
                nc.vector.tensor_scalar(dst[:, m, 512 * n:512 * (n + 1)], ps[:],
                                        b_sb[:, m:m + 1], None, op0=ADD)
            return emit

        def v_unit(tm, h0, h1):
            def emit():
                w = (h1 - h0) * CH
                ps = kv_ps.tile([128, 512], F32, tag="kv", name="ps_v")
                for j in range(NJ):
                    nc.tensor.matmul(ps[:, 0:w],
                                     xn[:, j, 128 * tm:128 * (tm + 1)],
                                     wv_sb[:, j, CH * h0:CH * h1],
                                     start=(j == 0), stop=(j == NJ - 1))
                nc.vector.tensor_copy(vT2[:, tm, h0:h1, 0:CH],
                                      ps[:, 0:w].rearrange("p (h c) -> p h c", c=CH))
            return emit

        # v in two groups: heads 0:2 (needed by pair 0 almost immediately,
        # small N=128 units) and heads 2:8 (needed from pair 1, wide N=384
        # units - fewer, PE-cheaper). k_m/q_m before pair m.
        fillers = []
        for tm in range(NTM):
            fillers.append(v_unit(tm, 0, 2))
        for n in range(2):
            fillers.append(kq_unit(wk_sb, bk_sb, k_sb, 1, n))
        for n in range(2):
            fillers.append(kq_unit(wq_sb, bq_sb, q_sb, 1, n))
        for tm in range(NTM):
            fillers.append(v_unit(tm, 2, H))
        for n in range(2):
            fillers.append(kq_unit(wk_sb, bk_sb, k_sb, 2, n))
        for n in range(2):
            fillers.append(kq_unit(wq_sb, bq_sb, q_sb, 2, n))
        for n in range(2):
            fillers.append(kq_unit(wk_sb, bk_sb, k_sb, 3, n))
        for n in range(2):
            fillers.append(kq_unit(wq_sb, bq_sb, q_sb, 3, n))

        # proj m=2 accumulates as a late filler: x-chunks any time, a-chunks
        # once the producing pair's evac has been emitted.
        psm2 = [None, None]

        def m2_unit(chunks, first):
            def emit():
                if first:
                    psm2[0] = kv_ps.tile([128, 512], F32, tag="kv", name="psm2a")
                    psm2[1] = kv_ps.tile([128, 512], F32, tag="kv", name="psm2b")
                for nh in range(2):
                    for jj in chunks:
                        rhs = (xbf if jj >= NJ else a_sb)
                        nc.tensor.matmul(psm2[nh][:],
                                         pw_sb[:, jj, 256:384],
                                         rhs[:, jj % NJ, 512 * nh:512 * (nh + 1)],
                                         start=(first and jj == chunks[0]), stop=False)
            return emit

        late = [(44, m2_unit([4, 5], True)), (47, m2_unit([6, 7], False)),
                (50, m2_unit([0], False)), (54, m2_unit([1], False)),
                (58, m2_unit([2], False))]

        pumped = [0]

        def pump(slot):
            if fillers:
                if pumped[0] < 20 or slot % 2 == 0:
                    fillers.pop(0)()
                    pumped[0] += 1
            elif late and slot >= late[0][0]:
                late.pop(0)[1]()
            elif slot < 44:
                warm_mm(1)

        # ---- q0 (k0 was interleaved with xn above) ----
        for n in range(2):
            kq_unit(wq_sb, bq_sb, q_sb, 0, n)()

        # ---- attention: software-pipelined (AV trails two sm globally) ----
        def emit_qk(p, n, sm):
            psw = qk_ps.tile([128, T], F32, tag="qk", name="psw")
            nc.tensor.matmul(psw[:, 0:512],
                             k_sb[0:64, p, 128 * sm:128 * (sm + 1)],
                             q_sb[0:64, p, 512 * n:512 * (n + 1)],
                             start=True, stop=True, tile_position=(0, 0))
            nc.tensor.matmul(psw[:, 512:1024],
                             k_sb[64:128, p, 128 * sm:128 * (sm + 1)],
                             q_sb[64:128, p, 512 * n:512 * (n + 1)],
                             start=True, stop=True, tile_position=(64, 0))
            ew = ewp.tile([128, 2, 512], BF, tag="ew", name="ew")
            nc.scalar.activation(ew[:], psw[:].rearrange("p (u t) -> p u t", u=2),
                                 AFT.Exp, bias=zero_sb[:], scale=EXP_SCALE)
            return ew

        def emit_evac(p, n, psa):
            # custom-DVE ops need partition-0 based APs on HW: stage row-sums
            for u in range(2):
                rs = rinvp.tile([64, 512], F32, tag="rs", name="rs")
                nc.vector.tensor_copy(rs[:], psa[u][64:128, :])
                rinv = rinvp.tile([64, 512], F32, tag="rinv", name="rinv")
                nc.vector.reciprocal_approx_fast(rinv[:], rs[:])
                nc.vector.tensor_mul(a_sb[64 * u:64 * (u + 1), p, 512 * n:512 * (n + 1)],
                                     psa[u][0:CH, :], rinv[:])

        blocks = [(p, n) for p in range(NJ) for n in range(2)]
        bpsa = {}
        pend = []

        def drain(auto_evac=True):
            bi, p, n, sm, ew = pend.pop(0)
            for u in range(2):
                nc.tensor.matmul(bpsa[bi][u][:],
                                 vT2[:, sm, 2 * p + u, :],
                                 ew[:, u, :],
                                 start=(sm == 0), stop=(sm == NTM - 1))
            if sm == NTM - 1 and auto_evac:
                emit_evac(p, n, bpsa[bi])
                return True
            return False

        for bi, (p, n) in enumerate(blocks):
            psa0 = av_ps.tile([128, 512], F32, tag="av", name="psa0")
            psa1 = av_ps.tile([128, 512], F32, tag="av", name="psa1")
            bpsa[bi] = [psa0, psa1]
            for sm in range(NTM):
                slot = 8 * bi + sm
                ew = emit_qk(p, n, sm)
                pend.append((bi, p, n, sm, ew))
                evd = False
                if len(pend) > 2:
                    evd = drain()
                if not evd:
                    pump(slot)

        # ---- tail ----
        drain()                      # AV(last, 6)
        lbi, lp, ln_, lsm, _lew = pend[0]
        drain(auto_evac=False)       # AV(last, 7)
        # m0 / m1: all chunks except j=3 (the last pair's a-tiles)
        psp0 = qk_ps.tile([128, T], F32, tag="qk", name="psp0")
        psp1 = qk_ps.tile([128, T], F32, tag="qk", name="psp1")
        pre = [4, 5, 6, 7, 0, 1, 2]
        for m in range(2):
            for nh in range(2):
                for jj in pre:
                    rhs = (xbf if jj >= NJ else a_sb)
                    nc.tensor.matmul([psp0, psp1][m][:, 512 * nh:512 * (nh + 1)],
                                     pw_sb[:, jj, 128 * m:128 * (m + 1)],
                                     rhs[:, jj % NJ, 512 * nh:512 * (nh + 1)],
                                     start=(jj == pre[0]), stop=False)
        emit_evac(lp, ln_, bpsa[lbi])
        psm3 = [av_ps.tile([128, 512], F32, tag="av", name="psm3a"),
                av_ps.tile([128, 512], F32, tag="av", name="psm3b")]
        for nh in range(2):
            for jj in pre:
                rhs = (xbf if jj >= NJ else a_sb)
                nc.tensor.matmul(psm3[nh][:],
                                 pw_sb[:, jj, 384:512],
                                 rhs[:, jj % NJ, 512 * nh:512 * (nh + 1)],
                                 start=(jj == pre[0]), stop=False)
        for m in range(2):
            for nh in range(2):
                nc.tensor.matmul([psp0, psp1][m][:, 512 * nh:512 * (nh + 1)],
                                 pw_sb[:, 3, 128 * m:128 * (m + 1)],
                                 a_sb[:, 3, 512 * nh:512 * (nh + 1)],
                                 start=False, stop=True)
        for nh in range(2):
            nc.tensor.matmul(psm2[nh][:],
                             pw_sb[:, 3, 256:384],
                             a_sb[:, 3, 512 * nh:512 * (nh + 1)],
                             start=False, stop=True)
        for nh in range(2):
            nc.tensor.matmul(psm3[nh][:],
                             pw_sb[:, 3, 384:512],
                             a_sb[:, 3, 512 * nh:512 * (nh + 1)],
                             start=False, stop=True)
        # evacuations: ScalarE (activation Copy + per-partition bias) and DVE
        # (tensor_scalar) drain in parallel; m3 runs on the freed kv banks.
        osb0 = outp.tile([128, T], F32, tag="osb", name="osb0")
        nc.scalar.activation(osb0[:], psp0[:], AFT.Identity, bias=pb_sb[:, 0:1], scale=1.0)
        nc.sync.dma_start(out_d[0:128, :], osb0[:])
        osb1 = outp.tile([128, T], F32, tag="osb", name="osb1")
        nc.vector.tensor_scalar(osb1[:], psp1[:], pb_sb[:, 1:2], None, op0=ADD)
        nc.sync.dma_start(out_d[128:256, :], osb1[:])
        for nh in range(2):
            osb2 = outp.tile([128, 512], F32, tag="osbh", name="osb2")
            nc.scalar.activation(osb2[:], psm2[nh][:], AFT.Identity, bias=pb_sb[:, 2:3], scale=1.0)
            nc.sync.dma_start(out_d[256:384, 512 * nh:512 * (nh + 1)], osb2[:])
        for nh in range(2):
            osb3 = outp.tile([128, 512], F32, tag="osbh", name="osb3")
            nc.vector.tensor_scalar(osb3[:], psm3[nh][:], pb_sb[:, 3:4], None, op0=ADD)
            nc.sync.dma_start(out_d[384:512, 512 * nh:512 * (nh + 1)], osb3[:])

    nc.compile()
    return nc


_NC_CACHE = {}


def get_nc():
    if "nc" not in _NC_CACHE:
        _NC_CACHE["nc"] = build_graph()
    return _NC_CACHE["nc"]


def make_in_maps(x, norm_scale, norm_bias, qkv_w, qkv_b, proj_w, proj_b):
    x = np.asarray(x, dtype=np.float32)
    B = x.shape[0]
    qr = np.asarray(qkv_w, np.float32).reshape(H, 3, CH, C)
    wq = np.ascontiguousarray(qr[:, 0].reshape(C, C).T).astype(BF_NP)
    wk = np.ascontiguousarray(qr[:, 1].reshape(C, C).T).astype(BF_NP)
    wv = np.ascontiguousarray(qr[:, 2].reshape(C, C).T).astype(BF_NP)
    br = np.asarray(qkv_b, np.float32).reshape(H, 3, CH)
    bq = np.ascontiguousarray(br[:, 0].reshape(C))
    bk = np.ascontiguousarray(br[:, 1].reshape(C))
    bv = np.ascontiguousarray(br[:, 2].reshape(C))
    pw_f = np.asarray(proj_w, np.float32)
    # residual folded in: [pw | I] so proj psum accumulates pw@a + x
    pw2 = np.ascontiguousarray(
        np.concatenate([pw_f.T, np.eye(C, dtype=np.float32)], axis=0)).astype(BF_NP)
    # v bias folded through proj: h = pw @ (a + bv) + pb = pw @ a + (pw@bv + pb)
    pb2 = np.asarray(proj_b, np.float32) + pw_f @ bv
    g8 = np.zeros((128, 8), np.float32)
    g8[np.arange(128), np.arange(128) // 16] = 1.0
    gt8 = np.ascontiguousarray(g8.T)
    g8s = g8 * np.float32(1.0 / GN_N)   # fold the 1/N of the group mean into g8
    cpack = np.zeros((128, 28), np.float32)
    cpack[:, 0:4] = np.asarray(norm_scale, np.float32).reshape(NJ, 128).T
    cpack[:, 4:8] = np.asarray(norm_bias, np.float32).reshape(NJ, 128).T
    cpack[:, 8:12] = bq.reshape(NJ, 128).T
    cpack[:, 12:16] = bk.reshape(NJ, 128).T
    cpack[:, 16:20] = pb2.reshape(NJ, 128).T
    cpack[:, 20:28] = g8s
    shared = dict(wq=wq, wk=wk, wv=wv, pw=pw2,
                  cpack=np.ascontiguousarray(cpack),
                  gt8=gt8)
    in_maps = []
    for i in range(B):
        m = dict(shared)
        m["xbf"] = np.ascontiguousarray(x[i].reshape(C, T).astype(BF_NP))
        in_maps.append(m)
    return in_maps


def kernel(x, norm_scale, norm_bias, qkv_w, qkv_b, proj_w, proj_b):
    x = np.asarray(x, dtype=np.float32)
    B, Cc, Hh, Ww = x.shape
    nc = get_nc()
    in_maps = make_in_maps(x, norm_scale, norm_bias, qkv_w, qkv_b, proj_w, proj_b)
    res = run_bass_kernel_spmd(nc, in_maps, core_ids=list(range(B)))
    out = np.stack([res.results[i]["out"] for i in range(B)])
    return out.reshape(B, Cc, Hh, Ww).astype(np.float32)
